# revision 35
# baseline (speedup 1.0000x reference)
"""LocalWindowAttention Trainium2 kernel (Bass/Tile), 8-core SPMD, v4.

Problem: x[B=4, S=4096, E=512] -> out[B, S, E]
  qkv = x @ W_qkv + b_qkv ; q,k,v = split(qkv)
  scores = (q @ k.T) / sqrt(E), banded mask |i-j| <= 64, softmax
  out = (attn @ v) @ W_out + b_out

Sharding: 8 cores = (batch b in 0..3) x (seq half h in 0..1). Each core owns
2048 query rows and loads a 64-row halo of x on each side (zero-padded at
sequence boundaries), computing q/k/v locally - no collectives.

v4 design notes (HW-trace driven):
  - q/k projections run in fp8e4m3 with DoubleRow perf mode: 2 matmuls per
    PSUM group instead of 4, each at 0.5 cycles/row. The error only enters
    through input quantization of x and Wq/Wk (the softmax is scale-
    sensitive to ~2-3% there, inside the rel-err budget); scores and
    everything downstream stay f32r/bf16.
  - v projection in bf16 (x loads once as fp8 for q/k and once as bf16 for
    v; there is no f32 copy of x at all). Total input DMA ~5.3MB vs 8.6MB,
    which un-bottlenecks the startup (first matmul needs only ~0.3MB).
  - f32r for scores/attended-weights/output projection (m512 f32r issues
    ~230ns vs bf16 ~265ns on HW); bf16 on the exp/transpose path where it
    wins (bf16 PE transpose ~75ns vs f32 ~190ns).
  - softmax normalization deferred: attention uses unnormalized exp(s);
    the output-projection PSUM->SBUF copy applies 1/rowsum as a per-
    partition activation scale (free). v's bias and the output bias
    collapse into b_all = b_v @ W_out + b_out, added on host.
"""

import sys

sys.path.insert(0, "/opt/trn_rl_repo")

import ml_dtypes
import numpy as np

import concourse.bass as bass  # noqa: F401  (registers types)
import concourse.tile as tile
from concourse import bacc, mybir
from concourse.bass_utils import run_bass_kernel_spmd

F32 = mybir.dt.float32
F32R = mybir.dt.float32r
BF16 = mybir.dt.bfloat16
FP8 = mybir.dt.float8e4
BF = ml_dtypes.bfloat16
F8 = ml_dtypes.float8_e4m3
DR = mybir.MatmulPerfMode.DoubleRow

B, S, E = 4, 4096, 512
WINDOW = 64
HALF = S // 2              # 2048 query rows per core
ROWS = HALF + 2 * WINDOW   # 2176 local rows incl. halo
EC = E // 128              # 4 contraction chunks
NT = HALF // 128           # 16 query subtiles per core
NDT = NT // 2              # 8 double tiles
# k covers all 2176 local rows; q only the owned 2048 (local row = x row - 64)
KSLICES = [(0, 512), (512, 512), (1024, 512), (1536, 384), (1920, 256)]
QSLICES = [(64, 512), (576, 512), (1088, 512), (1600, 512)]

# A/B feature flags (v4.2-equivalent baseline: all False)
import os
FLAG = lambda n, d="0": bool(int(os.environ.get(n, d)))
F_QKBF16 = FLAG("LWA_QKBF16")     # qT/kT stored bf16 instead of f32r
F_X1024 = FLAG("LWA_X1024")       # x8 stage split at 1024 + stage2 first
F_OSPLIT = FLAG("LWA_OSPLIT")     # out-DMA split sync/gpsimd
F_PTPAR = FLAG("LWA_PTPAR")       # pT0/pT2 parity double-buffer
F_OSTSPL = FLAG("LWA_OSTSPL")     # ost copies alternate scalar/vector
F_ATTSPL = FLAG("LWA_ATTSPL")     # attT copies alternate vector/scalar
F_BUFS3 = FLAG("LWA_BUFS3")       # attn pool bufs=3
F_DEPGATE = FLAG("LWA_DEPGATE")   # bulk DMAs gated on first k-copy

SW = 128.0   # fp8 weight pre-scale (power of 2: exact, avoids denormals)
SX = 16.0    # fp8 x pre-scale
SCL_K = 1.0 / (SW * SX)
SCL_Q = SCL_K / np.sqrt(E)

_NC_CACHE = {}


def _round_fp32r(x: np.ndarray) -> np.ndarray:
    """Round-to-nearest fp32 -> fp32r (11-bit mantissa) as walrus expects."""
    u = np.ascontiguousarray(x, dtype=np.float32).view(np.uint32)
    r = (u.astype(np.uint64) + 0x800) & 0xFFFFF000
    return np.ascontiguousarray(r.astype(np.uint32).view(np.float32))


def _build():
    nc = bacc.Bacc("TRN2", target_bir_lowering=False, debug=False, num_devices=8)

    x8_d = nc.dram_tensor("x8", [2, 128, 2, ROWS], FP8, kind="ExternalInput")
    xb_d = nc.dram_tensor("xb", [128, EC * ROWS], BF16, kind="ExternalInput")
    wk8_d = nc.dram_tensor("wk8", [128, 4 * 4 * 128], FP8, kind="ExternalInput")
    wq8_d = nc.dram_tensor("wq8", [128, 4 * 4 * 128], FP8, kind="ExternalInput")
    wv_d = nc.dram_tensor("wv", [128, 4 * 512], BF16, kind="ExternalInput")
    wo_d = nc.dram_tensor("wout", [128, 4 * 512], BF16, kind="ExternalInput")
    bqk_d = nc.dram_tensor("bqk", [128, 8], F32, kind="ExternalInput")
    mask_d = nc.dram_tensor("masks", [128, 3 * 256], BF16, kind="ExternalInput")
    idb_d = nc.dram_tensor("identb", [128, 128], BF16, kind="ExternalInput")
    out_d = nc.dram_tensor("out", [HALF, E], BF16, kind="ExternalOutput")

    ACT = mybir.ActivationFunctionType

    with tile.TileContext(nc) as tc:
        with (
            tc.tile_pool(name="const", bufs=1) as const,
            tc.tile_pool(name="big", bufs=1) as big,
            tc.tile_pool(name="attn", bufs=(3 if F_BUFS3 else 2)) as attn,
        ):
            # ---- persistent SBUF ----
            # fp8 x, e-chunk pairs stacked in a DoubleRow-shaped middle dim
            x8 = [big.tile([128, 2, ROWS], FP8, name=f"x8g{g}", tag=f"x8g{g}")
                  for g in range(2)]
            # bf16 x for the v projection, e-chunks packed in one tile
            xb = big.tile([128, EC * ROWS], BF16, name="xb", tag="xb")
            # fp8 k/q weights: [p, f, e, c] so (f, pair g) slices are 3D APs
            wk8 = big.tile([128, 4, 4, 128], FP8, name="wk8", tag="wk8")
            wq8 = big.tile([128, 4, 4, 128], FP8, name="wq8", tag="wq8")
            wv_sb = big.tile([128, 4 * 512], BF16, name="wv", tag="wv")
            wo_sb = big.tile([128, 4 * 512], BF16, name="wo", tag="wo")
            bq_sb = const.tile([128, 8], F32, name="bq", tag="bq")
            mask_sb = const.tile([128, 3 * 256], BF16, name="msk", tag="msk")
            idb_sb = const.tile([128, 128], BF16, name="idb", tag="idb")
            QKDT = BF16 if F_QKBF16 else F32R
            qT = [big.tile([128, HALF], QKDT, name=f"qT{f}", tag=f"qT{f}")
                  for f in range(EC)]
            kT = [big.tile([128, ROWS], QKDT, name=f"kT{f}", tag=f"kT{f}")
                  for f in range(EC)]
            v_sb = [big.tile([128, E], BF16, name=f"v{r}", tag=f"v{r}")
                    for r in range(ROWS // 128)]   # 17 natural-layout v chunks
            NPT = 2 if F_PTPAR else 1
            pT0 = [const.tile([128, 256], BF16, name=f"pT0_{p}", tag=f"pT0_{p}")
                   for p in range(NPT)]
            pT2 = [const.tile([128, 256], BF16, name=f"pT2_{p}", tag=f"pT2_{p}")
                   for p in range(NPT)]
            dep_sb = const.tile([1, 1], F32, name="dep", tag="dep")

            # ---- DMAs, startup-critical first. The sync queue issues in
            # order, so transfer priority follows emission order; big loads
            # that are needed later (xb, wo) go last. ----
            # startup-critical transfers spread across the three queues
            # with the earliest post-init DMA slots: scalar (~5.7us) takes
            # x8 stage-1, gpsimd (~6.0us) takes wk8, sync (~7.2us) takes
            # x8 stage-2 + wq8
            for g in range(2):
                nc.scalar.dma_start(out=x8[g][:, :, 0:512],
                                    in_=x8_d[g, :, :, 0:512])
            nc.gpsimd.dma_start(out=wk8[:], in_=wk8_d[:, :])
            for g in range(2):
                nc.sync.dma_start(out=x8[g][:, :, 512:ROWS],
                                  in_=x8_d[g, :, :, 512:ROWS])
            nc.sync.dma_start(out=wq8[:], in_=wq8_d[:, :])
            # small consts on the gpsimd (SWDGE) queue
            nc.gpsimd.dma_start(out=bq_sb[:], in_=bqk_d[:, :])
            nc.gpsimd.dma_start(out=mask_sb[:], in_=mask_d[:, :])
            nc.gpsimd.dma_start(out=idb_sb[:], in_=idb_d[:, :])
            # Bulk loads for later phases, gated on the first kT copy via a
            # dummy gpsimd read: DMA packets round-robin across all rings
            # regardless of issue order, so issuing these early starves the
            # startup-critical x8 stage-2 stream. xb splits into row-halves
            # per e-block so the v projection can start on the first half.
            if F_DEPGATE:
                nc.sync.dma_start(out=wv_sb[:], in_=wv_d[:, :])
                nc.sync.dma_start(out=xb[:], in_=xb_d[:, :])
                nc.sync.dma_start(out=wo_sb[:], in_=wo_d[:, :])
            else:
                nc.gpsimd.tensor_copy(dep_sb[:], kT[0][0:1, 0:1])
                nc.gpsimd.dma_start(out=wv_sb[:], in_=wv_d[:, :])
                XH = 1088
                for e in range(EC):
                    nc.gpsimd.dma_start(
                        out=xb[:, ROWS * e:ROWS * e + XH],
                        in_=xb_d[:, ROWS * e:ROWS * e + XH])
                for e in range(EC):
                    nc.gpsimd.dma_start(
                        out=xb[:, ROWS * e + XH:ROWS * (e + 1)],
                        in_=xb_d[:, ROWS * e + XH:ROWS * (e + 1)])
                nc.gpsimd.dma_start(out=wo_sb[:], in_=wo_d[:, :])

            # pT0 right half / pT2 left half must stay zero for the whole
            # kernel (written halves only, every double-tile)
            for p in range(NPT):
                nc.vector.memset(pT0[p][:], 0.0)
                nc.vector.memset(pT2[p][:], 0.0)

            # ---- phase 1: projections (PSUM pool scoped so phase 2 can
            #      use all 8 banks) ----
            with tc.tile_pool(name="pp", bufs=3, space="PSUM") as pp:
                # kT[f]: [feature, rows], fp8 DoubleRow (2 matmuls per group)
                for (r0, ns) in KSLICES:
                    for f in range(EC):
                        ps = pp.tile([128, 512], F32, name=f"pk{f}_{r0}", tag="pp")
                        for g in range(2):
                            nc.tensor.matmul(
                                ps[:, :ns],
                                wk8[:, f, 2 * g:2 * g + 2, :],
                                x8[g][:, :, r0:r0 + ns],
                                start=(g == 0), stop=(g == 1),
                                perf_mode=DR,
                            )
                        if f % 2 == 0:
                            nc.scalar.activation(
                                out=kT[f][:, r0:r0 + ns], in_=ps[:, :ns],
                                func=ACT.Identity, bias=bq_sb[:, f:f + 1],
                                scale=float(SCL_K),
                            )
                        else:
                            nc.vector.tensor_scalar(
                                out=kT[f][:, r0:r0 + ns], in0=ps[:, :ns],
                                scalar1=float(SCL_K),
                                scalar2=bq_sb[:, f:f + 1],
                                op0=mybir.AluOpType.mult,
                                op1=mybir.AluOpType.add,
                            )
                # qT[f]: [feature, 2048 owned rows], fp8 DoubleRow
                for j, (r0, ns) in enumerate(QSLICES):
                    for f in range(EC):
                        ps = pp.tile([128, 512], F32, name=f"pq{f}_{r0}", tag="pp")
                        for g in range(2):
                            nc.tensor.matmul(
                                ps[:, :ns],
                                wq8[:, f, 2 * g:2 * g + 2, :],
                                x8[g][:, :, r0:r0 + ns],
                                start=(g == 0), stop=(g == 1),
                                perf_mode=DR,
                            )
                        if f % 2 == 0:
                            nc.scalar.activation(
                                out=qT[f][:, 512 * j:512 * j + ns],
                                in_=ps[:, :ns],
                                func=ACT.Identity, bias=bq_sb[:, 4 + f:5 + f],
                                scale=float(SCL_Q),
                            )
                        else:
                            nc.vector.tensor_scalar(
                                out=qT[f][:, 512 * j:512 * j + ns],
                                in0=ps[:, :ns],
                                scalar1=float(SCL_Q),
                                scalar2=bq_sb[:, 4 + f:5 + f],
                                op0=mybir.AluOpType.mult,
                                op1=mybir.AluOpType.add,
                            )

                # v: [rows, feature] in bf16, no bias (folded into b_all)
                for r in range(ROWS // 128):
                    ps = pp.tile([128, 512], F32, name=f"pv{r}", tag="pp")
                    for e in range(EC):
                        nc.tensor.matmul(
                            ps[:],
                            xb[:, ROWS * e + 128 * r:ROWS * e + 128 * (r + 1)],
                            wv_sb[:, 512 * e:512 * (e + 1)],
                            start=(e == 0), stop=(e == EC - 1),
                        )
                    nc.vector.tensor_copy(v_sb[r][:], ps[:])

            # ---- phase 2: attention + output projection ----
            with (
                tc.tile_pool(name="ps_s", bufs=2, space="PSUM") as ps_s,
                tc.tile_pool(name="ps_t", bufs=2, space="PSUM") as ps_t,
                tc.tile_pool(name="ps_a", bufs=2, space="PSUM") as ps_a,
                tc.tile_pool(name="pp_out", bufs=2, space="PSUM") as pp_out,
            ):
                for T in range(NDT):
                    pT1 = attn.tile([128, 256], BF16, name=f"pT1_{T}", tag="pT1")
                    rds = []
                    for s_half in (0, 1):
                        t = 2 * T + s_half
                        # scores [128 q, 256 keys]
                        ps = ps_s.tile([128, 256], F32, name=f"s{t}", tag="ps_s")
                        for e in range(EC):
                            nc.tensor.matmul(
                                ps[:],
                                qT[e][:, 128 * t:128 * (t + 1)],
                                kT[e][:, 128 * t:128 * t + 256],
                                start=(e == 0), stop=(e == EC - 1),
                            )
                        # additive band mask (0 / -1e30), exp w/ fused rowsum
                        mi = 0 if t == 0 else (2 if t == NT - 1 else 1)
                        sm = attn.tile([128, 256], BF16, name=f"sm{t}", tag="sm")
                        nc.vector.tensor_add(
                            sm[:], ps[:], mask_sb[:, 256 * mi:256 * (mi + 1)])
                        pe_t = attn.tile([128, 256], BF16, name=f"pe{t}", tag="pe")
                        rs = attn.tile([128, 1], F32, name=f"rs{t}", tag="rs")
                        nc.scalar.activation(out=pe_t[:], in_=sm[:], func=ACT.Exp,
                                             accum_out=rs[:])
                        rd = attn.tile([128, 1], F32, name=f"rd{t}", tag="rd")
                        nc.vector.reciprocal(rd[:], rs[:])
                        rds.append(rd)
                        # transpose both halves onto pT tiles (bf16: 1 cyc/row)
                        for half in (0, 1):
                            pt_ps = ps_t.tile([128, 128], BF16,
                                              name=f"tp{t}_{half}", tag="ps_t")
                            nc.tensor.transpose(
                                pt_ps[:], pe_t[:, 128 * half:128 * (half + 1)],
                                idb_sb[:])
                            if s_half == 0 and half == 0:
                                dst = pT0[T % NPT][:, 0:128]
                            elif s_half == 0 and half == 1:
                                dst = pT1[:, 0:128]
                            elif s_half == 1 and half == 0:
                                dst = pT1[:, 128:256]
                            else:
                                dst = pT2[T % NPT][:, 128:256]
                            nc.vector.tensor_copy(dst, pt_ps[:])

                    # attended (unnormalized), transposed: attT[e', q(256)]
                    pTs = (pT0[T % NPT], pT1, pT2[T % NPT])
                    attT = []
                    for e in range(EC):
                        pa = ps_a.tile([128, 256], F32, name=f"pa{T}_{e}",
                                       tag="ps_a")
                        for kc in range(3):
                            nc.tensor.matmul(
                                pa[:],
                                v_sb[2 * T + kc][:, 128 * e:128 * (e + 1)],
                                pTs[kc][:],
                                start=(kc == 0), stop=(kc == 2),
                            )
                        at = attn.tile([128, 256], BF16, name=f"attT{T}_{e}",
                                       tag=f"attT{e}")
                        if e % 2 == 0 or not F_ATTSPL:
                            nc.vector.tensor_copy(at[:], pa[:])
                        else:
                            nc.scalar.copy(at[:], pa[:])
                        attT.append(at)

                    # output projection per 128-query subtile; the PSUM->SBUF
                    # copy applies the deferred softmax normalization
                    for s_half in (0, 1):
                        t = 2 * T + s_half
                        po = pp_out.tile([128, 512], F32, name=f"po{t}",
                                         tag="pp_out")
                        for e in range(EC):
                            nc.tensor.matmul(
                                po[:],
                                attT[e][:, 128 * s_half:128 * (s_half + 1)],
                                wo_sb[:, 512 * e:512 * (e + 1)],
                                start=(e == 0), stop=(e == EC - 1),
                            )
                        ost = attn.tile([128, 512], BF16, name=f"ost{t}",
                                        tag="ost")
                        if s_half == 0 or not F_OSTSPL:
                            nc.scalar.activation(out=ost[:], in_=po[:],
                                                 func=ACT.Copy,
                                                 scale=rds[s_half][:])
                        else:
                            nc.vector.tensor_scalar_mul(
                                ost[:], po[:], rds[s_half][:])
                        if F_OSPLIT and t >= 12:
                            for p4 in range(4):
                                nc.sync.dma_start(
                                    out=out_d[128 * t + 32 * p4:
                                              128 * t + 32 * (p4 + 1), :],
                                    in_=ost[32 * p4:32 * (p4 + 1), :])
                        else:
                            nc.sync.dma_start(
                                out=out_d[128 * t:128 * (t + 1), :],
                                in_=ost[:])
    nc.compile()
    return nc


def _get_nc():
    key = (F_QKBF16, F_X1024, F_OSPLIT, F_PTPAR, F_OSTSPL, F_ATTSPL,
           F_BUFS3, F_DEPGATE)
    if key not in _NC_CACHE:
        _NC_CACHE[key] = _build()
    return _NC_CACHE[key]


def _prep_shared(W_qkv, b_qkv, W_out, b_out):
    scale = np.float32(1.0 / np.sqrt(E))
    w = np.array(W_qkv, dtype=np.float32, copy=True)
    wq3 = w.reshape(EC, 128, 3 * E)
    # k and q fp8 blocks f-major: [128, (f, e, 128)], pre-scaled by SW so
    # the small W entries stay in fp8 normal range (scale undone in the
    # PSUM->SBUF copy)
    k8 = (wq3[:, :, E:2 * E].reshape(EC, 128, EC, 128)
          .transpose(1, 2, 0, 3).reshape(128, 2048) * SW).astype(F8)
    q8 = (wq3[:, :, 0:E].reshape(EC, 128, EC, 128)
          .transpose(1, 2, 0, 3).reshape(128, 2048) * SW).astype(F8)
    # v bf16 block e-major: [128, (e, 512)]
    wv = (wq3[:, :, 2 * E:3 * E].transpose(1, 0, 2)
          .reshape(128, 2048)).astype(BF)

    wo_host = np.ascontiguousarray(
        np.asarray(W_out, np.float32).reshape(EC, 128, E)
        .transpose(1, 0, 2).reshape(128, 4 * 512)).astype(BF)

    b = np.asarray(b_qkv, np.float32)
    bqk = np.empty((128, 8), np.float32)
    for f in range(4):
        bqk[:, f] = b[E + 128 * f:E + 128 * (f + 1)]            # k bias
        bqk[:, 4 + f] = b[128 * f:128 * (f + 1)] * scale        # q bias (scaled)

    # v bias and out bias collapse into one output-space row vector
    b_all = (b[2 * E:3 * E].astype(np.float64) @
             np.asarray(W_out, np.float64) +
             np.asarray(b_out, np.float64)).astype(np.float32)

    shared = {
        "wk8": np.ascontiguousarray(k8),
        "wq8": np.ascontiguousarray(q8),
        "wv": np.ascontiguousarray(wv),
        "wout": wo_host,
        "bqk": np.ascontiguousarray(bqk),
        "identb": np.eye(128, dtype=np.float32).astype(BF),
    }
    return shared, b_all


def _masks_for(h: int) -> np.ndarray:
    """Additive masks: 0 where attendable, -1e30 outside the band (or past
    the sequence boundary). Columns: [t0 mask | interior mask | t15 mask]."""
    ii = np.arange(128)[:, None]
    jj = np.arange(256)[None, :]
    band = (jj - ii >= 0) & (jj - ii <= 2 * WINDOW)
    m_mid = band
    m_t0 = band & (jj >= 64) if h == 0 else band
    m_t15 = band & (jj < 192) if h == 1 else band
    stacked = np.concatenate([m_t0, m_mid, m_t15], axis=1)
    return np.ascontiguousarray(
        np.where(stacked, np.float32(0.0), np.float32(-1e30))).astype(BF)


def _install_ntff_shim():
    """The agent image's antenv lacks axon_hooks; synthesize it from the
    boot module's ctypes NTFF driver so trace=True can capture HW timing."""
    import types
    if "antenv.axon_hooks" in sys.modules:
        return
    try:
        from trn_agent_boot.trn_boot import _ntff_profile_via_ctypes
        hook = _ntff_profile_via_ctypes("/opt/axon/libaxon_pjrt.so")
    except Exception:
        hook = None
    mod = types.ModuleType("antenv.axon_hooks")
    mod.get_axon_ntff_profile_hook = lambda: hook
    mod.set_axon_ntff_profile_hook = lambda h: None
    sys.modules["antenv.axon_hooks"] = mod
    # avoid S3 artifact upload attempts during local profile processing
    try:
        from concourse import bass_utils as _bu
        _bu.upload_artifacts = lambda tmpdir: tmpdir
    except Exception:
        pass


def kernel(x, W_qkv, b_qkv, W_out, b_out, _trace=False):
    x = np.asarray(x, dtype=np.float32)
    nc = _get_nc()
    shared, b_all = _prep_shared(W_qkv, b_qkv, W_out, b_out)
    masks = [_masks_for(0), _masks_for(1)]

    in_maps = []
    for core in range(8):
        b, h = divmod(core, 2)
        lo = h * HALF - WINDOW
        hi = lo + ROWS
        xh = np.zeros((ROWS, E), dtype=np.float32)
        s0, s1 = max(lo, 0), min(hi, S)
        xh[s0 - lo:s1 - lo] = x[b, s0:s1]
        xT = np.ascontiguousarray(xh.T)
        x8 = (xT * SX).astype(F8).reshape(2, 2, 128, ROWS).transpose(0, 2, 1, 3)
        xbp = xT.astype(BF).reshape(EC, 128, ROWS).transpose(1, 0, 2) \
            .reshape(128, EC * ROWS)
        in_maps.append({
            "x8": np.ascontiguousarray(x8),
            "xb": np.ascontiguousarray(xbp),
            "masks": masks[h],
            **shared,
        })

    kwargs = {}
    if _trace:
        _install_ntff_shim()
        kwargs = dict(trace=True, trace_cores=[0])
    res = run_bass_kernel_spmd(nc, in_maps, core_ids=list(range(8)), **kwargs)

    out = np.empty((B, S, E), dtype=np.float32)
    for core in range(8):
        b, h = divmod(core, 2)
        out[b, h * HALF:(h + 1) * HALF] = \
            res.results[core]["out"].astype(np.float32) + b_all[None, :]
    if _trace:
        return out, res
    return out


# revision 36
# speedup vs baseline: 1.0524x; 1.0524x over previous
"""LocalWindowAttention Trainium2 kernel (Bass/Tile), 8-core SPMD, v4.

Problem: x[B=4, S=4096, E=512] -> out[B, S, E]
  qkv = x @ W_qkv + b_qkv ; q,k,v = split(qkv)
  scores = (q @ k.T) / sqrt(E), banded mask |i-j| <= 64, softmax
  out = (attn @ v) @ W_out + b_out

Sharding: 8 cores = (batch b in 0..3) x (seq half h in 0..1). Each core owns
2048 query rows and loads a 64-row halo of x on each side (zero-padded at
sequence boundaries), computing q/k/v locally - no collectives.

v4 design notes (HW-trace driven):
  - q/k projections run in fp8e4m3 with DoubleRow perf mode: 2 matmuls per
    PSUM group instead of 4, each at 0.5 cycles/row. The error only enters
    through input quantization of x and Wq/Wk (the softmax is scale-
    sensitive to ~2-3% there, inside the rel-err budget); scores and
    everything downstream stay f32r/bf16.
  - v projection in bf16 (x loads once as fp8 for q/k and once as bf16 for
    v; there is no f32 copy of x at all). Total input DMA ~5.3MB vs 8.6MB,
    which un-bottlenecks the startup (first matmul needs only ~0.3MB).
  - f32r for scores/attended-weights/output projection (m512 f32r issues
    ~230ns vs bf16 ~265ns on HW); bf16 on the exp/transpose path where it
    wins (bf16 PE transpose ~75ns vs f32 ~190ns).
  - softmax normalization deferred: attention uses unnormalized exp(s);
    the output-projection PSUM->SBUF copy applies 1/rowsum as a per-
    partition activation scale (free). v's bias and the output bias
    collapse into b_all = b_v @ W_out + b_out, added on host.
"""

import sys

sys.path.insert(0, "/opt/trn_rl_repo")

import ml_dtypes
import numpy as np

import concourse.bass as bass  # noqa: F401  (registers types)
import concourse.tile as tile
from concourse import bacc, mybir
from concourse.bass_utils import run_bass_kernel_spmd

F32 = mybir.dt.float32
F32R = mybir.dt.float32r
BF16 = mybir.dt.bfloat16
FP8 = mybir.dt.float8e4
BF = ml_dtypes.bfloat16
F8 = ml_dtypes.float8_e4m3
DR = mybir.MatmulPerfMode.DoubleRow

B, S, E = 4, 4096, 512
WINDOW = 64
HALF = S // 2              # 2048 query rows per core
ROWS = HALF + 2 * WINDOW   # 2176 local rows incl. halo
EC = E // 128              # 4 contraction chunks
NT = HALF // 128           # 16 query subtiles per core
NDT = NT // 2              # 8 double tiles
# k covers all 2176 local rows; q only the owned 2048 (local row = x row - 64)
KSLICES = [(0, 512), (512, 512), (1024, 512), (1536, 384), (1920, 256)]
QSLICES = [(64, 512), (576, 512), (1088, 512), (1600, 512)]

# A/B feature flags (v4.2-equivalent baseline: all False)
import os
FLAG = lambda n, d="0": bool(int(os.environ.get(n, d)))
F_QKBF16 = FLAG("LWA_QKBF16")     # qT/kT stored bf16 instead of f32r
F_X1024 = FLAG("LWA_X1024")       # x8 stage split at 1024 + stage2 first
F_OSPLIT = FLAG("LWA_OSPLIT")     # out-DMA split sync/gpsimd
F_PTPAR = FLAG("LWA_PTPAR")       # pT0/pT2 parity double-buffer
F_OSTSPL = FLAG("LWA_OSTSPL")     # ost copies alternate scalar/vector
F_ATTSPL = FLAG("LWA_ATTSPL")     # attT copies alternate vector/scalar
F_BUFS3 = FLAG("LWA_BUFS3")       # attn pool bufs=3
F_DEPGATE = FLAG("LWA_DEPGATE")   # bulk DMAs gated on first k-copy

SW = 128.0   # fp8 weight pre-scale (power of 2: exact, avoids denormals)
SX = 16.0    # fp8 x pre-scale
SCL_K = 1.0 / (SW * SX)
SCL_Q = SCL_K / np.sqrt(E)

_NC_CACHE = {}


def _round_fp32r(x: np.ndarray) -> np.ndarray:
    """Round-to-nearest fp32 -> fp32r (11-bit mantissa) as walrus expects."""
    u = np.ascontiguousarray(x, dtype=np.float32).view(np.uint32)
    r = (u.astype(np.uint64) + 0x800) & 0xFFFFF000
    return np.ascontiguousarray(r.astype(np.uint32).view(np.float32))


def _build():
    nc = bacc.Bacc("TRN2", target_bir_lowering=False, debug=False, num_devices=8)

    x8_d = nc.dram_tensor("x8", [2, 128, 2, ROWS], FP8, kind="ExternalInput")
    xb_d = nc.dram_tensor("xb", [128, EC * ROWS], BF16, kind="ExternalInput")
    wk8_d = nc.dram_tensor("wk8", [128, 4 * 4 * 128], FP8, kind="ExternalInput")
    wq8_d = nc.dram_tensor("wq8", [128, 4 * 4 * 128], FP8, kind="ExternalInput")
    wv_d = nc.dram_tensor("wv", [128, 4 * 512], BF16, kind="ExternalInput")
    wo_d = nc.dram_tensor("wout", [128, 4 * 512], BF16, kind="ExternalInput")
    bqk_d = nc.dram_tensor("bqk", [128, 8], F32, kind="ExternalInput")
    mask_d = nc.dram_tensor("masks", [128, 3 * 256], BF16, kind="ExternalInput")
    idb_d = nc.dram_tensor("identb", [128, 128], BF16, kind="ExternalInput")
    out_d = nc.dram_tensor("out", [HALF, E], BF16, kind="ExternalOutput")

    ACT = mybir.ActivationFunctionType

    with tile.TileContext(nc) as tc:
        with (
            tc.tile_pool(name="const", bufs=1) as const,
            tc.tile_pool(name="big", bufs=1) as big,
            tc.tile_pool(name="attn", bufs=(3 if F_BUFS3 else 2)) as attn,
        ):
            # ---- persistent SBUF ----
            # fp8 x, e-chunk pairs stacked in a DoubleRow-shaped middle dim
            x8 = [big.tile([128, 2, ROWS], FP8, name=f"x8g{g}", tag=f"x8g{g}")
                  for g in range(2)]
            # bf16 x for the v projection, e-chunks packed in one tile
            xb = big.tile([128, EC * ROWS], BF16, name="xb", tag="xb")
            # fp8 k/q weights: [p, f, e, c] so (f, pair g) slices are 3D APs
            wk8 = big.tile([128, 4, 4, 128], FP8, name="wk8", tag="wk8")
            wq8 = big.tile([128, 4, 4, 128], FP8, name="wq8", tag="wq8")
            wv_sb = big.tile([128, 4 * 512], BF16, name="wv", tag="wv")
            wo_sb = big.tile([128, 4 * 512], BF16, name="wo", tag="wo")
            bq_sb = const.tile([128, 8], F32, name="bq", tag="bq")
            mask_sb = const.tile([128, 3 * 256], BF16, name="msk", tag="msk")
            idb_sb = const.tile([128, 128], BF16, name="idb", tag="idb")
            QKDT = BF16 if F_QKBF16 else F32R
            qT = [big.tile([128, HALF], QKDT, name=f"qT{f}", tag=f"qT{f}")
                  for f in range(EC)]
            kT = [big.tile([128, ROWS], QKDT, name=f"kT{f}", tag=f"kT{f}")
                  for f in range(EC)]
            v_sb = [big.tile([128, E], BF16, name=f"v{r}", tag=f"v{r}")
                    for r in range(ROWS // 128)]   # 17 natural-layout v chunks
            NPT = 2 if F_PTPAR else 1
            pT0 = [const.tile([128, 256], BF16, name=f"pT0_{p}", tag=f"pT0_{p}")
                   for p in range(NPT)]
            pT2 = [const.tile([128, 256], BF16, name=f"pT2_{p}", tag=f"pT2_{p}")
                   for p in range(NPT)]
            dep_sb = const.tile([1, 1], F32, name="dep", tag="dep")

            # ---- DMAs, startup-critical first. The sync queue issues in
            # order, so transfer priority follows emission order; big loads
            # that are needed later (xb, wo) go last. ----
            # x8 stage-1 issues from the scalar queue, which finishes its
            # init ~1.2us before sync's first DMA slot
            for g in range(2):
                nc.scalar.dma_start(out=x8[g][:, :, 0:512],
                                    in_=x8_d[g, :, :, 0:512])
            nc.sync.dma_start(out=wk8[:], in_=wk8_d[:, :])
            for g in range(2):
                nc.sync.dma_start(out=x8[g][:, :, 512:ROWS],
                                  in_=x8_d[g, :, :, 512:ROWS])
            nc.sync.dma_start(out=wq8[:], in_=wq8_d[:, :])
            # small consts on the gpsimd (SWDGE) queue
            nc.gpsimd.dma_start(out=bq_sb[:], in_=bqk_d[:, :])
            nc.gpsimd.dma_start(out=mask_sb[:], in_=mask_d[:, :])
            nc.gpsimd.dma_start(out=idb_sb[:], in_=idb_d[:, :])
            # Bulk loads for later phases, gated on the first kT copy via a
            # dummy gpsimd read: DMA packets round-robin across all rings
            # regardless of issue order, so issuing these early starves the
            # startup-critical x8 stage-2 stream. xb splits into row-halves
            # per e-block so the v projection can start on the first half.
            if F_DEPGATE:
                nc.sync.dma_start(out=wv_sb[:], in_=wv_d[:, :])
                nc.sync.dma_start(out=xb[:], in_=xb_d[:, :])
                nc.sync.dma_start(out=wo_sb[:], in_=wo_d[:, :])
            else:
                nc.gpsimd.tensor_copy(dep_sb[:], kT[0][0:1, 0:1])
                nc.gpsimd.dma_start(out=wv_sb[:], in_=wv_d[:, :])
                XH = 1088
                for e in range(EC):
                    nc.gpsimd.dma_start(
                        out=xb[:, ROWS * e:ROWS * e + XH],
                        in_=xb_d[:, ROWS * e:ROWS * e + XH])
                for e in range(EC):
                    nc.gpsimd.dma_start(
                        out=xb[:, ROWS * e + XH:ROWS * (e + 1)],
                        in_=xb_d[:, ROWS * e + XH:ROWS * (e + 1)])
                nc.gpsimd.dma_start(out=wo_sb[:], in_=wo_d[:, :])

            # pT0 right half / pT2 left half must stay zero for the whole
            # kernel (written halves only, every double-tile)
            for p in range(NPT):
                nc.vector.memset(pT0[p][:], 0.0)
                nc.vector.memset(pT2[p][:], 0.0)

            # ---- phase 1: projections (PSUM pool scoped so phase 2 can
            #      use all 8 banks) ----
            with tc.tile_pool(name="pp", bufs=3, space="PSUM") as pp:
                # kT[f]: [feature, rows], fp8 DoubleRow (2 matmuls per group)
                for (r0, ns) in KSLICES:
                    for f in range(EC):
                        ps = pp.tile([128, 512], F32, name=f"pk{f}_{r0}", tag="pp")
                        for g in range(2):
                            nc.tensor.matmul(
                                ps[:, :ns],
                                wk8[:, f, 2 * g:2 * g + 2, :],
                                x8[g][:, :, r0:r0 + ns],
                                start=(g == 0), stop=(g == 1),
                                perf_mode=DR,
                            )
                        if f % 2 == 0:
                            nc.scalar.activation(
                                out=kT[f][:, r0:r0 + ns], in_=ps[:, :ns],
                                func=ACT.Identity, bias=bq_sb[:, f:f + 1],
                                scale=float(SCL_K),
                            )
                        else:
                            nc.vector.tensor_scalar(
                                out=kT[f][:, r0:r0 + ns], in0=ps[:, :ns],
                                scalar1=float(SCL_K),
                                scalar2=bq_sb[:, f:f + 1],
                                op0=mybir.AluOpType.mult,
                                op1=mybir.AluOpType.add,
                            )
                # qT[f]: [feature, 2048 owned rows], fp8 DoubleRow
                for j, (r0, ns) in enumerate(QSLICES):
                    for f in range(EC):
                        ps = pp.tile([128, 512], F32, name=f"pq{f}_{r0}", tag="pp")
                        for g in range(2):
                            nc.tensor.matmul(
                                ps[:, :ns],
                                wq8[:, f, 2 * g:2 * g + 2, :],
                                x8[g][:, :, r0:r0 + ns],
                                start=(g == 0), stop=(g == 1),
                                perf_mode=DR,
                            )
                        if f % 2 == 0:
                            nc.scalar.activation(
                                out=qT[f][:, 512 * j:512 * j + ns],
                                in_=ps[:, :ns],
                                func=ACT.Identity, bias=bq_sb[:, 4 + f:5 + f],
                                scale=float(SCL_Q),
                            )
                        else:
                            nc.vector.tensor_scalar(
                                out=qT[f][:, 512 * j:512 * j + ns],
                                in0=ps[:, :ns],
                                scalar1=float(SCL_Q),
                                scalar2=bq_sb[:, 4 + f:5 + f],
                                op0=mybir.AluOpType.mult,
                                op1=mybir.AluOpType.add,
                            )

                # v: [rows, feature] in bf16, no bias (folded into b_all)
                for r in range(ROWS // 128):
                    ps = pp.tile([128, 512], F32, name=f"pv{r}", tag="pp")
                    for e in range(EC):
                        nc.tensor.matmul(
                            ps[:],
                            xb[:, ROWS * e + 128 * r:ROWS * e + 128 * (r + 1)],
                            wv_sb[:, 512 * e:512 * (e + 1)],
                            start=(e == 0), stop=(e == EC - 1),
                        )
                    nc.vector.tensor_copy(v_sb[r][:], ps[:])

            # ---- phase 2: attention + output projection ----
            with (
                tc.tile_pool(name="ps_s", bufs=2, space="PSUM") as ps_s,
                tc.tile_pool(name="ps_t", bufs=2, space="PSUM") as ps_t,
                tc.tile_pool(name="ps_a", bufs=2, space="PSUM") as ps_a,
                tc.tile_pool(name="pp_out", bufs=2, space="PSUM") as pp_out,
            ):
                for T in range(NDT):
                    pT1 = attn.tile([128, 256], BF16, name=f"pT1_{T}", tag="pT1")
                    rds = []
                    for s_half in (0, 1):
                        t = 2 * T + s_half
                        # scores [128 q, 256 keys]
                        ps = ps_s.tile([128, 256], F32, name=f"s{t}", tag="ps_s")
                        for e in range(EC):
                            nc.tensor.matmul(
                                ps[:],
                                qT[e][:, 128 * t:128 * (t + 1)],
                                kT[e][:, 128 * t:128 * t + 256],
                                start=(e == 0), stop=(e == EC - 1),
                            )
                        # additive band mask (0 / -1e30), exp w/ fused rowsum
                        mi = 0 if t == 0 else (2 if t == NT - 1 else 1)
                        sm = attn.tile([128, 256], BF16, name=f"sm{t}", tag="sm")
                        nc.vector.tensor_add(
                            sm[:], ps[:], mask_sb[:, 256 * mi:256 * (mi + 1)])
                        pe_t = attn.tile([128, 256], BF16, name=f"pe{t}", tag="pe")
                        rs = attn.tile([128, 1], F32, name=f"rs{t}", tag="rs")
                        nc.scalar.activation(out=pe_t[:], in_=sm[:], func=ACT.Exp,
                                             accum_out=rs[:])
                        rd = attn.tile([128, 1], F32, name=f"rd{t}", tag="rd")
                        nc.vector.reciprocal(rd[:], rs[:])
                        rds.append(rd)
                        # transpose both halves onto pT tiles (bf16: 1 cyc/row)
                        for half in (0, 1):
                            pt_ps = ps_t.tile([128, 128], BF16,
                                              name=f"tp{t}_{half}", tag="ps_t")
                            nc.tensor.transpose(
                                pt_ps[:], pe_t[:, 128 * half:128 * (half + 1)],
                                idb_sb[:])
                            if s_half == 0 and half == 0:
                                dst = pT0[T % NPT][:, 0:128]
                            elif s_half == 0 and half == 1:
                                dst = pT1[:, 0:128]
                            elif s_half == 1 and half == 0:
                                dst = pT1[:, 128:256]
                            else:
                                dst = pT2[T % NPT][:, 128:256]
                            nc.vector.tensor_copy(dst, pt_ps[:])

                    # attended (unnormalized), transposed: attT[e', q(256)]
                    pTs = (pT0[T % NPT], pT1, pT2[T % NPT])
                    attT = []
                    for e in range(EC):
                        pa = ps_a.tile([128, 256], F32, name=f"pa{T}_{e}",
                                       tag="ps_a")
                        for kc in range(3):
                            nc.tensor.matmul(
                                pa[:],
                                v_sb[2 * T + kc][:, 128 * e:128 * (e + 1)],
                                pTs[kc][:],
                                start=(kc == 0), stop=(kc == 2),
                            )
                        at = attn.tile([128, 256], BF16, name=f"attT{T}_{e}",
                                       tag=f"attT{e}")
                        if e % 2 == 0 or not F_ATTSPL:
                            nc.vector.tensor_copy(at[:], pa[:])
                        else:
                            nc.scalar.copy(at[:], pa[:])
                        attT.append(at)

                    # output projection per 128-query subtile; the PSUM->SBUF
                    # copy applies the deferred softmax normalization
                    for s_half in (0, 1):
                        t = 2 * T + s_half
                        po = pp_out.tile([128, 512], F32, name=f"po{t}",
                                         tag="pp_out")
                        for e in range(EC):
                            nc.tensor.matmul(
                                po[:],
                                attT[e][:, 128 * s_half:128 * (s_half + 1)],
                                wo_sb[:, 512 * e:512 * (e + 1)],
                                start=(e == 0), stop=(e == EC - 1),
                            )
                        ost = attn.tile([128, 512], BF16, name=f"ost{t}",
                                        tag="ost")
                        if s_half == 0 or not F_OSTSPL:
                            nc.scalar.activation(out=ost[:], in_=po[:],
                                                 func=ACT.Copy,
                                                 scale=rds[s_half][:])
                        else:
                            nc.vector.tensor_scalar_mul(
                                ost[:], po[:], rds[s_half][:])
                        if F_OSPLIT and t >= 12:
                            for p4 in range(4):
                                nc.sync.dma_start(
                                    out=out_d[128 * t + 32 * p4:
                                              128 * t + 32 * (p4 + 1), :],
                                    in_=ost[32 * p4:32 * (p4 + 1), :])
                        else:
                            nc.sync.dma_start(
                                out=out_d[128 * t:128 * (t + 1), :],
                                in_=ost[:])
    nc.compile()
    return nc


def _get_nc():
    key = (F_QKBF16, F_X1024, F_OSPLIT, F_PTPAR, F_OSTSPL, F_ATTSPL,
           F_BUFS3, F_DEPGATE)
    if key not in _NC_CACHE:
        _NC_CACHE[key] = _build()
    return _NC_CACHE[key]


def _prep_shared(W_qkv, b_qkv, W_out, b_out):
    scale = np.float32(1.0 / np.sqrt(E))
    w = np.array(W_qkv, dtype=np.float32, copy=True)
    wq3 = w.reshape(EC, 128, 3 * E)
    # k and q fp8 blocks f-major: [128, (f, e, 128)], pre-scaled by SW so
    # the small W entries stay in fp8 normal range (scale undone in the
    # PSUM->SBUF copy)
    k8 = (wq3[:, :, E:2 * E].reshape(EC, 128, EC, 128)
          .transpose(1, 2, 0, 3).reshape(128, 2048) * SW).astype(F8)
    q8 = (wq3[:, :, 0:E].reshape(EC, 128, EC, 128)
          .transpose(1, 2, 0, 3).reshape(128, 2048) * SW).astype(F8)
    # v bf16 block e-major: [128, (e, 512)]
    wv = (wq3[:, :, 2 * E:3 * E].transpose(1, 0, 2)
          .reshape(128, 2048)).astype(BF)

    wo_host = np.ascontiguousarray(
        np.asarray(W_out, np.float32).reshape(EC, 128, E)
        .transpose(1, 0, 2).reshape(128, 4 * 512)).astype(BF)

    b = np.asarray(b_qkv, np.float32)
    bqk = np.empty((128, 8), np.float32)
    for f in range(4):
        bqk[:, f] = b[E + 128 * f:E + 128 * (f + 1)]            # k bias
        bqk[:, 4 + f] = b[128 * f:128 * (f + 1)] * scale        # q bias (scaled)

    # v bias and out bias collapse into one output-space row vector
    b_all = (b[2 * E:3 * E].astype(np.float64) @
             np.asarray(W_out, np.float64) +
             np.asarray(b_out, np.float64)).astype(np.float32)

    shared = {
        "wk8": np.ascontiguousarray(k8),
        "wq8": np.ascontiguousarray(q8),
        "wv": np.ascontiguousarray(wv),
        "wout": wo_host,
        "bqk": np.ascontiguousarray(bqk),
        "identb": np.eye(128, dtype=np.float32).astype(BF),
    }
    return shared, b_all


def _masks_for(h: int) -> np.ndarray:
    """Additive masks: 0 where attendable, -1e30 outside the band (or past
    the sequence boundary). Columns: [t0 mask | interior mask | t15 mask]."""
    ii = np.arange(128)[:, None]
    jj = np.arange(256)[None, :]
    band = (jj - ii >= 0) & (jj - ii <= 2 * WINDOW)
    m_mid = band
    m_t0 = band & (jj >= 64) if h == 0 else band
    m_t15 = band & (jj < 192) if h == 1 else band
    stacked = np.concatenate([m_t0, m_mid, m_t15], axis=1)
    return np.ascontiguousarray(
        np.where(stacked, np.float32(0.0), np.float32(-1e30))).astype(BF)


def _install_ntff_shim():
    """The agent image's antenv lacks axon_hooks; synthesize it from the
    boot module's ctypes NTFF driver so trace=True can capture HW timing."""
    import types
    if "antenv.axon_hooks" in sys.modules:
        return
    try:
        from trn_agent_boot.trn_boot import _ntff_profile_via_ctypes
        hook = _ntff_profile_via_ctypes("/opt/axon/libaxon_pjrt.so")
    except Exception:
        hook = None
    mod = types.ModuleType("antenv.axon_hooks")
    mod.get_axon_ntff_profile_hook = lambda: hook
    mod.set_axon_ntff_profile_hook = lambda h: None
    sys.modules["antenv.axon_hooks"] = mod
    # avoid S3 artifact upload attempts during local profile processing
    try:
        from concourse import bass_utils as _bu
        _bu.upload_artifacts = lambda tmpdir: tmpdir
    except Exception:
        pass


def kernel(x, W_qkv, b_qkv, W_out, b_out, _trace=False):
    x = np.asarray(x, dtype=np.float32)
    nc = _get_nc()
    shared, b_all = _prep_shared(W_qkv, b_qkv, W_out, b_out)
    masks = [_masks_for(0), _masks_for(1)]

    in_maps = []
    for core in range(8):
        b, h = divmod(core, 2)
        lo = h * HALF - WINDOW
        hi = lo + ROWS
        xh = np.zeros((ROWS, E), dtype=np.float32)
        s0, s1 = max(lo, 0), min(hi, S)
        xh[s0 - lo:s1 - lo] = x[b, s0:s1]
        xT = np.ascontiguousarray(xh.T)
        x8 = (xT * SX).astype(F8).reshape(2, 2, 128, ROWS).transpose(0, 2, 1, 3)
        xbp = xT.astype(BF).reshape(EC, 128, ROWS).transpose(1, 0, 2) \
            .reshape(128, EC * ROWS)
        in_maps.append({
            "x8": np.ascontiguousarray(x8),
            "xb": np.ascontiguousarray(xbp),
            "masks": masks[h],
            **shared,
        })

    kwargs = {}
    if _trace:
        _install_ntff_shim()
        kwargs = dict(trace=True, trace_cores=[0])
    res = run_bass_kernel_spmd(nc, in_maps, core_ids=list(range(8)), **kwargs)

    out = np.empty((B, S, E), dtype=np.float32)
    for core in range(8):
        b, h = divmod(core, 2)
        out[b, h * HALF:(h + 1) * HALF] = \
            res.results[core]["out"].astype(np.float32) + b_all[None, :]
    if _trace:
        return out, res
    return out
